# revision 1
# baseline (speedup 1.0000x reference)
"""Trainium2 Bass kernel for the LocalizeModule retrieval problem.

Computation (reference):
    f  = relu(feat @ W1.T + b1) @ W2.T + b2        # [F, H]
    k  = keyword @ Wk.T + bk                       # [K, H]
    out = (cos_sim(k, f) + 1) * 0.49               # [K, F]

Sharding across 8 cores: frames (F) are sharded; each core computes the
frame MLP for its F/8 shard, the full keyword projection (replicated),
and the score tile [K, F/8].  No collectives needed; host concatenates
the 8 output shards along F.

On-chip layout is "transposed" (H on partitions) throughout so that all
matmuls contract over the partition dimension and biases are
per-partition.  Row norms are computed with a ones-vector matmul over
squared activations; the (1/norm) factors are applied in the score
epilogue (per-partition for keywords, replicated-broadcast for frames).
All matmuls run in bf16 with fp32 PSUM accumulation.
"""

import numpy as np
import ml_dtypes

import concourse.bass as bass  # noqa: F401  (bass types used via tile/bacc)
import concourse.mybir as mybir
import concourse.tile as tile
from concourse import bacc
from concourse.bass_utils import run_bass_kernel_spmd

P = 128
H = 1024
F = 8192
K = 4096
NCORES = 8
FS = F // NCORES          # 1024 frames per core
HO = H // P               # 8 partition chunks of the hidden dim
NCH = 512                 # matmul moving/free chunk (one PSUM bank of fp32)
F_CHUNKS = FS // NCH      # 2
K_CHUNKS = K // NCH       # 8
K_TILES = K // P          # 32
EPS = 1e-8
OUT_SCALE = 0.49

BF16 = mybir.dt.bfloat16
FP8 = mybir.dt.float8e4
F32 = mybir.dt.float32
AF = mybir.ActivationFunctionType
ALU = mybir.AluOpType

import os as _os

# which GEMM stages run in fp8-e4m3 with DoubleRow (2x PE throughput)
_FP8_STAGES = frozenset(
    s for s in _os.environ.get("KERNEL_FP8", "").split(",") if s
)
MLP_FP8 = "mlp" in _FP8_STAGES
KLIN_FP8 = "klin" in _FP8_STAGES
SCORE_FP8 = "score" in _FP8_STAGES

_CACHE = {}

LAST_EXEC_NS = None
LAST_RESULTS = None


def _enable_ldw_opt():
    """Let walrus elide duplicate LDWEIGHTS (we order matmuls so consecutive
    ones share the stationary operand).  Wraps the compile-command runner."""
    import concourse.bass_utils as _bu

    if getattr(_bu, "_ldw_opt_wrapped", False):
        return
    _orig = _bu.run_command

    def _patched(argv, **kw):
        argv = [
            "--enable-ldw-opt=true" if a == "--enable-ldw-opt=false" else a
            for a in argv
        ]
        return _orig(argv, **kw)

    _bu.run_command = _patched
    _bu._ldw_opt_wrapped = True


if _os.environ.get("KERNEL_LDW_OPT", "0") == "1":
    _enable_ldw_opt()


def _emit(tc, io):
    nc = tc.nc
    featT_d, kwT_d, w1t_d, w2t_d, wkt_d, b1_d, b2_d, bk_d, out_d = io

    import contextlib

    MLP_DT = FP8 if MLP_FP8 else BF16
    KLIN_DT = FP8 if KLIN_FP8 else BF16
    SC_DT = FP8 if SCORE_FP8 else BF16

    def mm_accum(ps, lhs_t, lhs_sl, rhs_t, rhs_sl, fp8):
        """Accumulate over the HO axis; fp8 stages use DoubleRow pairs."""
        step = 2 if fp8 else 1
        n = HO // step
        pm = mybir.MatmulPerfMode.DoubleRow if fp8 else None
        for i in range(n):
            ho = i * step
            if fp8:
                lhs = lhs_t[:, ho:ho + 2, lhs_sl]
                rhs = rhs_t[:, ho:ho + 2, rhs_sl]
            else:
                lhs = lhs_t[:, ho, lhs_sl]
                rhs = rhs_t[:, ho, rhs_sl]
            nc.tensor.matmul(
                ps, lhs, rhs, start=(i == 0), stop=(i == n - 1), perf_mode=pm
            )

    with contextlib.ExitStack() as ctx:
        const = ctx.enter_context(tc.tile_pool(name="const", bufs=1))
        psum = ctx.enter_context(tc.tile_pool(name="psum", bufs=1, space="PSUM"))

        # ---- persistent SBUF tensors -------------------------------------
        w2t_s = const.tile([P, HO, H], MLP_DT)
        wkt_s = const.tile([P, HO, H], KLIN_DT)
        b1_s = const.tile([P, HO], F32)
        b2_s = const.tile([P, HO], F32)
        bk_s = const.tile([P, HO], F32)
        ones_s = const.tile([P, 1], BF16)
        hT_s = const.tile([P, HO, FS], MLP_DT)     # relu(W1 @ featT + b1)
        fT_s = const.tile([P, HO, FS], SC_DT)      # projected frames, transposed
        kT_s = const.tile([P, HO, K], SC_DT)       # projected keywords, transposed
        fnorm2_s = const.tile([1, FS], F32)
        rfn_row = const.tile([1, FS], F32)
        rfn_b = const.tile([P, FS], F32)           # 0.49 / ||f_j||, replicated on partitions
        knp_raw = const.tile([P, K_TILES], F32)    # ||k_i||^2, partition-major per k-tile
        rkn_p = const.tile([P, K_TILES], F32)      # 1 / ||k_i||, partition-major per k-tile

        bias049_s = const.tile([P, 1], F32)
        nc.vector.memset(bias049_s[:], OUT_SCALE)
        nc.vector.memset(ones_s[:], 1.0)

        # ---- MLP layer 1 (scoped inputs: featT, W1T) ---------------------
        with tc.tile_pool(name="mlp_in", bufs=1) as mlp_in:
            featT_s = mlp_in.tile([P, HO, FS], MLP_DT)
            w1t_s = mlp_in.tile([P, HO, H], MLP_DT)
            # split the critical first loads into per-ho pieces so they fan
            # out across DMA queues (a single 2 MB DMA is queue-bound)
            for ho in range(HO):
                nc.sync.dma_start(w1t_s[:, ho], w1t_d[:, ho])
                nc.sync.dma_start(featT_s[:, ho, 0:NCH], featT_d[:, ho, 0:NCH])
            for ho in range(HO):
                nc.sync.dma_start(featT_s[:, ho, NCH:FS], featT_d[:, ho, NCH:FS])
            # non-critical constants load behind them
            nc.sync.dma_start(b1_s[:], b1_d[:])
            nc.sync.dma_start(b2_s[:], b2_d[:])
            nc.sync.dma_start(bk_s[:], bk_d[:])
            for ho in range(HO):
                nc.sync.dma_start(w2t_s[:, ho], w2t_d[:, ho])
                nc.sync.dma_start(wkt_s[:, ho], wkt_d[:, ho])

            for c in range(F_CHUNKS):
                for mo in range(HO):
                    h1_ps = psum.tile([P, NCH], F32, tag="mm", bufs=6, name="h1_ps")
                    mm_accum(h1_ps[:], w1t_s, slice(mo * P, (mo + 1) * P),
                             featT_s, slice(c * NCH, (c + 1) * NCH), MLP_FP8)
                    nc.scalar.activation(
                        hT_s[:, mo, c * NCH:(c + 1) * NCH],
                        h1_ps[:],
                        AF.Relu,
                        bias=b1_s[:, mo:mo + 1],
                        scale=1.0,
                    )

        with tc.tile_pool(name="work", bufs=1) as work:

            def new_sq(name="sq"):
                return work.tile([P, HO, NCH], BF16, tag="sq", bufs=2, name=name)

            def emit_tree_sum(sq):
                """Pairwise-tree DVE sum of sq over the HO axis (depth 3)."""
                tmps = []
                for i in range(HO // 2):
                    tmp = work.tile([P, NCH], BF16, tag="tsum", bufs=4, name="tsum")
                    nc.vector.tensor_tensor(
                        tmp[:], sq[:, 2 * i, :], sq[:, 2 * i + 1, :], ALU.add
                    )
                    tmps.append(tmp)
                nc.vector.tensor_tensor(tmps[0][:], tmps[0][:], tmps[1][:], ALU.add)
                nc.vector.tensor_tensor(tmps[2][:], tmps[2][:], tmps[3][:], ALU.add)
                ssum = work.tile([P, NCH], BF16, tag="sqs", bufs=3, name="ssum")
                nc.vector.tensor_tensor(ssum[:], tmps[0][:], tmps[2][:], ALU.add)
                return ssum

            # ---- MLP layer 2 + frame squares -----------------------------
            f_ssums = []
            for c in range(F_CHUNKS):
                sq = new_sq("sqf")
                for mo in range(HO):
                    f2_ps = psum.tile([P, NCH], F32, tag="mm", bufs=6, name="f2_ps")
                    mm_accum(f2_ps[:], w2t_s, slice(mo * P, (mo + 1) * P),
                             hT_s, slice(c * NCH, (c + 1) * NCH), MLP_FP8)
                    f_sl = fT_s[:, mo, c * NCH:(c + 1) * NCH]
                    nc.vector.tensor_scalar_add(f_sl, f2_ps[:], b2_s[:, mo:mo + 1])
                    nc.scalar.square(sq[:, mo, :], f_sl)
                f_ssums.append(emit_tree_sum(sq))

            def emit_fnorm_mm(ssum, c):
                # frame norms, free-major: ones as lhsT -> psum [1, NCH]
                nf_ps = psum.tile([1, NCH], F32, tag="nrm", bufs=1, name="nf_ps")
                nc.tensor.matmul(nf_ps[:], ones_s[:], ssum[:], start=True, stop=True)
                nc.scalar.copy(fnorm2_s[0:1, c * NCH:(c + 1) * NCH], nf_ps[:])

            def emit_fnorm_chain():
                # 0.49/max(sqrt(n),eps) == 1/max(sqrt(n/0.49^2), eps/0.49);
                # fold 0.49 into the sqrt scale, and split the single-lane
                # reciprocal into pieces the DVE pipeline can absorb
                nc.scalar.activation(
                    fnorm2_s[:], fnorm2_s[:], AF.Sqrt,
                    bias=0.0, scale=1.0 / (OUT_SCALE * OUT_SCALE),
                )
                nc.vector.tensor_scalar_max(fnorm2_s[:], fnorm2_s[:], EPS / OUT_SCALE)
                for r in range(HO):
                    nc.vector.reciprocal(
                        rfn_row[0:1, r * P:(r + 1) * P],
                        fnorm2_s[0:1, r * P:(r + 1) * P],
                    )
                nc.gpsimd.partition_broadcast(rfn_b[:], rfn_row[:])

            def emit_knorm_mms(ssum, c):
                # keyword norms, partition-major: squares as lhsT -> psum [P, 1]
                for sub in range(NCH // P):
                    t = c * (NCH // P) + sub
                    nk_ps = psum.tile([P, 1], F32, tag="cn", bufs=1, name="nk_ps")
                    nc.tensor.matmul(
                        nk_ps[:],
                        ssum[:, sub * P:(sub + 1) * P],
                        ones_s[:],
                        start=True,
                        stop=True,
                    )
                    nc.scalar.copy(knp_raw[:, t:t + 1], nk_ps[:])
                # incremental 1/max(sqrt, eps) on this chunk's 4 columns so the
                # score epilogue never waits on a long end-of-phase chain
                cols = slice(c * (NCH // P), (c + 1) * (NCH // P))
                nc.scalar.sqrt(knp_raw[:, cols], knp_raw[:, cols])
                nc.vector.tensor_scalar_max(knp_raw[:, cols], knp_raw[:, cols], EPS)
                nc.vector.reciprocal(rkn_p[:, cols], knp_raw[:, cols])

            # ---- keyword projection + keyword norms ----------------------
            def emit_fnorms():
                for c in range(F_CHUNKS):
                    emit_fnorm_mm(f_ssums[c], c)
                emit_fnorm_chain()

            pending = [emit_fnorms]
            for c in range(K_CHUNKS):
                kw_t = work.tile([P, HO, NCH], KLIN_DT, tag="kw", bufs=2, name="kw_t")
                nc.sync.dma_start(kw_t[:], kwT_d[:, :, c * NCH:(c + 1) * NCH])
                sqk = new_sq("sqk")
                for mo in range(HO):
                    kk_ps = psum.tile([P, NCH], F32, tag="mm", bufs=6, name="kk_ps")
                    mm_accum(kk_ps[:], wkt_s, slice(mo * P, (mo + 1) * P),
                             kw_t, slice(0, NCH), KLIN_FP8)
                    k_sl = kT_s[:, mo, c * NCH:(c + 1) * NCH]
                    nc.vector.tensor_scalar_add(k_sl, kk_ps[:], bk_s[:, mo:mo + 1])
                    nc.scalar.square(sqk[:, mo, :], k_sl)
                ssum_k = emit_tree_sum(sqk)
                # delayed by one chunk so the PE never waits on ACT/DVE
                for fn in pending:
                    fn()
                pending = [lambda s=ssum_k, cc=c: emit_knorm_mms(s, cc)]


            # ---- score GEMM + epilogue -----------------------------------
            step = 2 if SCORE_FP8 else 1
            n_acc = HO // step
            pm = mybir.MatmulPerfMode.DoubleRow if SCORE_FP8 else None
            for t in range(K_TILES):
                s_pss = [
                    psum.tile([P, NCH], F32, tag="mm", bufs=6, name="s_ps")
                    for _ in range(F_CHUNKS)
                ]
                for i in range(n_acc):
                    ho = i * step
                    if SCORE_FP8:
                        lhs = kT_s[:, ho:ho + 2, t * P:(t + 1) * P]
                    else:
                        lhs = kT_s[:, ho, t * P:(t + 1) * P]
                    for c in range(F_CHUNKS):
                        rhs = (fT_s[:, ho:ho + 2, c * NCH:(c + 1) * NCH]
                               if SCORE_FP8
                               else fT_s[:, ho, c * NCH:(c + 1) * NCH])
                        nc.tensor.matmul(
                            s_pss[c][:], lhs, rhs,
                            start=(i == 0), stop=(i == n_acc - 1),
                            perf_mode=pm,
                        )
                if t == 0:
                    # last keyword-norm minis, emitted here so the PE never
                    # idles at the klin->score boundary
                    for fn in pending:
                        fn()
                    pending = []
                for c in range(F_CHUNKS):
                    s_ps = s_pss[c]
                    stage = work.tile([P, NCH], F32, tag="stage", bufs=6, name="stage")
                    nc.vector.tensor_tensor(
                        stage[:],
                        s_ps[:],
                        rfn_b[:, c * NCH:(c + 1) * NCH],
                        ALU.mult,
                    )
                    out_t = work.tile([P, NCH], F32, tag="out_t", bufs=6, name="out_t")
                    nc.scalar.activation(
                        out_t[:],
                        stage[:],
                        AF.Identity,
                        bias=bias049_s[:, 0:1],
                        scale=rkn_p[:, t:t + 1],
                    )
                    nc.sync.dma_start(
                        out_d[t * P:(t + 1) * P, c * NCH:(c + 1) * NCH],
                        out_t[:],
                    )


def build():
    """Build + compile the (core-agnostic) Bass program once."""
    key = ("nc", MLP_FP8, KLIN_FP8, SCORE_FP8)
    if key in _CACHE:
        return _CACHE[key]
    MLP_DT = FP8 if MLP_FP8 else BF16
    KLIN_DT = FP8 if KLIN_FP8 else BF16
    nc = bacc.Bacc(
        "TRN2",
        target_bir_lowering=False,
        debug=False,
        enable_asserts=False,
        num_devices=NCORES,
    )
    featT_d = nc.dram_tensor("featT", [P, HO, FS], MLP_DT, kind="ExternalInput").ap()
    kwT_d = nc.dram_tensor("kwT", [P, HO, K], KLIN_DT, kind="ExternalInput").ap()
    w1t_d = nc.dram_tensor("w1t", [P, HO, H], MLP_DT, kind="ExternalInput").ap()
    w2t_d = nc.dram_tensor("w2t", [P, HO, H], MLP_DT, kind="ExternalInput").ap()
    wkt_d = nc.dram_tensor("wkt", [P, HO, H], KLIN_DT, kind="ExternalInput").ap()
    b1_d = nc.dram_tensor("b1t", [P, HO], F32, kind="ExternalInput").ap()
    b2_d = nc.dram_tensor("b2t", [P, HO], F32, kind="ExternalInput").ap()
    bk_d = nc.dram_tensor("bkt", [P, HO], F32, kind="ExternalInput").ap()
    out_d = nc.dram_tensor("out", [K, FS], F32, kind="ExternalOutput").ap()

    io = (featT_d, kwT_d, w1t_d, w2t_d, wkt_d, b1_d, b2_d, bk_d, out_d)
    with tile.TileContext(nc) as tc:
        _emit(tc, io)
    nc.compile()
    _CACHE[key] = nc
    return nc


def _part_tile(a):
    """[D0, rest...] with D0 = o*P + p  ->  [P, D0//P, rest...]"""
    d0 = a.shape[0]
    return np.ascontiguousarray(
        a.reshape(d0 // P, P, *a.shape[1:]).swapaxes(0, 1)
    )


def make_in_maps(feat, keyword, W1, b1, W2, b2, Wk, bk):
    mlp_np = ml_dtypes.float8_e4m3 if MLP_FP8 else ml_dtypes.bfloat16
    klin_np = ml_dtypes.float8_e4m3 if KLIN_FP8 else ml_dtypes.bfloat16
    feat = np.asarray(feat, np.float32)
    keyword = np.asarray(keyword, np.float32)
    kwT = _part_tile(np.ascontiguousarray(keyword.T)).astype(klin_np)   # [P, HO, K]
    w1t = _part_tile(np.ascontiguousarray(np.asarray(W1, np.float32).T)).astype(mlp_np)
    w2t = _part_tile(np.ascontiguousarray(np.asarray(W2, np.float32).T)).astype(mlp_np)
    wkt = _part_tile(np.ascontiguousarray(np.asarray(Wk, np.float32).T)).astype(klin_np)
    b1t = _part_tile(np.asarray(b1, np.float32))                        # [P, HO]
    b2t = _part_tile(np.asarray(b2, np.float32))
    bkt = _part_tile(np.asarray(bk, np.float32))

    in_maps = []
    for c in range(NCORES):
        featT_c = _part_tile(
            np.ascontiguousarray(feat[c * FS:(c + 1) * FS, :].T)
        ).astype(mlp_np)                                                # [P, HO, FS]
        in_maps.append({
            "featT": featT_c,
            "kwT": kwT,
            "w1t": w1t,
            "w2t": w2t,
            "wkt": wkt,
            "b1t": b1t,
            "b2t": b2t,
            "bkt": bkt,
        })
    return in_maps


def kernel(feat, keyword, W1, b1, W2, b2, Wk, bk, _trace=False):
    global LAST_EXEC_NS, LAST_RESULTS
    nc = build()
    in_maps = make_in_maps(feat, keyword, W1, b1, W2, b2, Wk, bk)
    res = run_bass_kernel_spmd(
        nc,
        in_maps,
        core_ids=list(range(NCORES)),
        trace=_trace,
    )
    LAST_EXEC_NS = res.exec_time_ns
    LAST_RESULTS = res
    out = np.concatenate([res.results[c]["out"] for c in range(NCORES)], axis=1)
    return np.ascontiguousarray(out.astype(np.float32))



# revision 2
# speedup vs baseline: 1.7130x; 1.7130x over previous
"""Trainium2 Bass kernel for the LocalizeModule retrieval problem.

Computation (reference):
    f  = relu(feat @ W1.T + b1) @ W2.T + b2        # [F, H]
    k  = keyword @ Wk.T + bk                       # [K, H]
    out = (cos_sim(k, f) + 1) * 0.49               # [K, F]

Sharding across 8 cores (v2):
  * frames (F) sharded for the MLP: each core projects its F/8 frames;
  * keywords (K) sharded for the keyword projection: each core projects
    K/8 keywords, normalizes them (k-hat * 64), quantizes to fp8-e4m3,
    and AllGathers the full normalized keyword matrix (0.5 MB/rank on
    the TOPSP/SDMA collective path, fully overlapped with the MLP);
  * score GEMM per core: [FS, K] = fT.T @ k8_all in fp8 DoubleRow with
    the frame tile stationary (best LDWEIGHTS amortization).  Output is
    the TRANSPOSED score shard [FS, K]; the host concatenates out_c.T
    along F.

On-chip layout keeps H on partitions throughout so all matmuls contract
over the partition dim.  Frame norms land partition-major via a
ones-vector matmul, so the whole score epilogue is one ScalarE op per
PSUM bank: out = rfn * psum + 0.49 with rfn = (0.49/64)/max(||f||,eps).
"""

import numpy as np
import ml_dtypes

import concourse.bass as bass  # noqa: F401  (bass types used via tile/bacc)
import concourse.mybir as mybir
import concourse.tile as tile
from concourse import bacc
from concourse.bass_utils import run_bass_kernel_spmd

P = 128
H = 1024
F = 8192
K = 4096
NCORES = 8
FS = F // NCORES          # 1024 frames per core
KS = K // NCORES          # 512 keywords per core
HO = H // P               # 8 partition chunks of the hidden dim
NCH = 512                 # matmul moving/free chunk (one PSUM bank of fp32)
F_CHUNKS = FS // NCH      # 2
F_TILES = FS // P         # 8
K_CHUNKS = K // NCH       # 8
EPS = 1e-8
OUT_SCALE = 0.49
KSCALE = 64.0             # pre-scale of normalized keywords into fp8 range

BF16 = mybir.dt.bfloat16
FP8 = mybir.dt.float8e4
F32 = mybir.dt.float32
AF = mybir.ActivationFunctionType
ALU = mybir.AluOpType

_CACHE = {}

LAST_EXEC_NS = None
LAST_RESULTS = None


def _emit(tc, io):
    nc = tc.nc
    featT_d, kwT_d, w1t_d, w2t_d, wkt_d, b1_d, b2_d, bk_d, out_d = io

    import contextlib

    with contextlib.ExitStack() as ctx:
        const = ctx.enter_context(tc.tile_pool(name="const", bufs=1))
        psum = ctx.enter_context(tc.tile_pool(name="psum", bufs=1, space="PSUM"))
        dram = ctx.enter_context(tc.tile_pool(name="dram", bufs=1, space="DRAM"))

        # ---- persistent SBUF tensors -------------------------------------
        wkt_s = const.tile([P, HO, H], BF16)
        w2t_s = const.tile([P, HO, H], BF16)
        b1_s = const.tile([P, HO], F32)
        b2_s = const.tile([P, HO], F32)
        bk_s = const.tile([P, HO], F32)
        ones_s = const.tile([P, 1], BF16)
        bias049_s = const.tile([P, 1], F32)
        kbf_s = const.tile([P, HO, KS], BF16)     # projected keyword shard
        k8_all = const.tile([P, HO, K], FP8)      # gathered normalized keywords
        hT_s = const.tile([P, HO, FS], BF16)      # relu(W1 @ featT + b1)
        f8_s = const.tile([P, HO, FS], FP8)       # projected frames, fp8
        rkn_b = const.tile([P, KS], F32)          # KSCALE/||k||, bcast on partitions
        nf_raw = const.tile([P, F_TILES], F32)    # ||f||^2, partition-major
        rfn_p = const.tile([P, F_TILES], F32)     # (0.49/KSCALE)/||f||, partition-major

        nc.vector.memset(bias049_s[:], OUT_SCALE)
        nc.vector.memset(ones_s[:], 1.0)

        # DRAM bounce buffers for the keyword AllGather
        cc_in = dram.tile([P, HO, KS], FP8)
        cc_out = dram.tile([NCORES, P, HO, KS], FP8, addr_space="Shared")

        def mm_accum(ps, lhs_t, lhs_sl, rhs_t, rhs_sl):
            for ho in range(HO):
                nc.tensor.matmul(
                    ps, lhs_t[:, ho, lhs_sl], rhs_t[:, ho, rhs_sl],
                    start=(ho == 0), stop=(ho == HO - 1),
                )

        with tc.tile_pool(name="work", bufs=1) as work:

            def emit_tree_sum(sq, width):
                """Pairwise-tree DVE sum of sq[:, ho, :] over the HO axis."""
                tmps = []
                for i in range(HO // 2):
                    tmp = work.tile([P, width], BF16, tag="tsum", bufs=4, name="tsum")
                    nc.vector.tensor_tensor(
                        tmp[:], sq[:, 2 * i, :width], sq[:, 2 * i + 1, :width], ALU.add
                    )
                    tmps.append(tmp)
                nc.vector.tensor_tensor(tmps[0][:], tmps[0][:], tmps[1][:], ALU.add)
                nc.vector.tensor_tensor(tmps[2][:], tmps[2][:], tmps[3][:], ALU.add)
                ssum = work.tile([P, width], BF16, tag="sqs", bufs=3, name="ssum")
                nc.vector.tensor_tensor(ssum[:], tmps[0][:], tmps[2][:], ALU.add)
                return ssum

            # ---- phase K: keyword projection on this core's K/8 shard ----
            kw_s = work.tile([P, HO, KS], BF16, tag="kw", bufs=1, name="kw_s")
            sqk_s = work.tile([P, HO, KS], BF16, tag="sqk", bufs=1, name="sqk_s")
            for ho in range(HO):
                nc.sync.dma_start(kw_s[:, ho], kwT_d[:, ho])
                nc.sync.dma_start(wkt_s[:, ho], wkt_d[:, ho])
            nc.sync.dma_start(bk_s[:], bk_d[:])
            nc.sync.dma_start(b1_s[:], b1_d[:])
            nc.sync.dma_start(b2_s[:], b2_d[:])

            with tc.tile_pool(name="mlp_in", bufs=1) as mlp_in:
                featT_s = mlp_in.tile([P, HO, FS], BF16)
                w1t_s = mlp_in.tile([P, HO, H], BF16)
                # MLP1 inputs queue behind the keyword-projection inputs
                for ho in range(HO):
                    nc.sync.dma_start(w1t_s[:, ho], w1t_d[:, ho])
                    nc.sync.dma_start(featT_s[:, ho, 0:NCH], featT_d[:, ho, 0:NCH])
                for ho in range(HO):
                    nc.sync.dma_start(featT_s[:, ho, NCH:FS], featT_d[:, ho, NCH:FS])
                for ho in range(HO):
                    nc.sync.dma_start(w2t_s[:, ho], w2t_d[:, ho])

                for mo in range(HO):
                    kk_ps = psum.tile([P, KS], F32, tag="mm", bufs=6, name="kk_ps")
                    mm_accum(kk_ps[:], wkt_s, slice(mo * P, (mo + 1) * P),
                             kw_s, slice(0, KS))
                    nc.vector.tensor_scalar_add(
                        kbf_s[:, mo, :], kk_ps[:], bk_s[:, mo:mo + 1]
                    )
                    nc.scalar.activation(
                        sqk_s[:, mo, :], kk_ps[:], AF.Square,
                        bias=bk_s[:, mo:mo + 1], scale=1.0,
                    )

                # keyword norms (free-major) -> normalize -> fp8 -> AllGather
                ssum_k = emit_tree_sum(sqk_s, KS)
                nk_ps = psum.tile([1, KS], F32, tag="cn", bufs=1, name="nk_ps")
                nc.tensor.matmul(nk_ps[:], ones_s[:], ssum_k[:], start=True, stop=True)
                knr = work.tile([1, KS], F32, tag="knr", bufs=1, name="knr")
                # sqrt(nk / KSCALE^2) = ||k||/KSCALE; clamp; reciprocal
                nc.scalar.activation(
                    knr[:], nk_ps[:], AF.Sqrt, bias=0.0, scale=1.0 / (KSCALE * KSCALE)
                )
                nc.vector.tensor_scalar_max(knr[:], knr[:], EPS / KSCALE)
                nc.vector.reciprocal(knr[:], knr[:])
                nc.gpsimd.partition_broadcast(rkn_b[:], knr[:])
                k8_stage = work.tile([P, HO, KS], FP8, tag="k8st", bufs=1, name="k8st")
                for ho in range(HO):
                    nc.vector.tensor_tensor(
                        k8_stage[:, ho, :], kbf_s[:, ho, :], rkn_b[:], ALU.mult
                    )
                nc.sync.dma_start(cc_in[:], k8_stage[:])
                nc.gpsimd.collective_compute(
                    "AllGather",
                    mybir.AluOpType.bypass,
                    replica_groups=[list(range(NCORES))],
                    ins=[cc_in.opt()],
                    outs=[cc_out.opt()],
                )
                for r in range(NCORES):
                    nc.sync.dma_start(
                        k8_all[:, :, r * KS:(r + 1) * KS], cc_out[r]
                    )

                # ---- MLP layer 1 ------------------------------------------
                for c in range(F_CHUNKS):
                    for mo in range(HO):
                        h1_ps = psum.tile([P, NCH], F32, tag="mm", bufs=6, name="h1_ps")
                        mm_accum(h1_ps[:], w1t_s, slice(mo * P, (mo + 1) * P),
                                 featT_s, slice(c * NCH, (c + 1) * NCH))
                        nc.scalar.activation(
                            hT_s[:, mo, c * NCH:(c + 1) * NCH],
                            h1_ps[:],
                            AF.Relu,
                            bias=b1_s[:, mo:mo + 1],
                            scale=1.0,
                        )

            # ---- MLP layer 2 + frame norms (partition-major) -------------
            for c in range(F_CHUNKS):
                sqf = work.tile([P, HO, NCH], BF16, tag="sqf", bufs=2, name="sqf")
                for mo in range(HO):
                    f2_ps = psum.tile([P, NCH], F32, tag="mm", bufs=6, name="f2_ps")
                    mm_accum(f2_ps[:], w2t_s, slice(mo * P, (mo + 1) * P),
                             hT_s, slice(c * NCH, (c + 1) * NCH))
                    nc.vector.tensor_scalar_add(
                        f8_s[:, mo, c * NCH:(c + 1) * NCH], f2_ps[:],
                        b2_s[:, mo:mo + 1],
                    )
                    nc.scalar.activation(
                        sqf[:, mo, :], f2_ps[:], AF.Square,
                        bias=b2_s[:, mo:mo + 1], scale=1.0,
                    )
                ssum_f = emit_tree_sum(sqf, NCH)
                for sub in range(NCH // P):
                    i = c * (NCH // P) + sub
                    nf_ps = psum.tile([P, 1], F32, tag="nrm", bufs=1, name="nf_ps")
                    nc.tensor.matmul(
                        nf_ps[:], ssum_f[:, sub * P:(sub + 1) * P], ones_s[:],
                        start=True, stop=True,
                    )
                    nc.scalar.copy(nf_raw[:, i:i + 1], nf_ps[:])

            # rfn = (0.49/KSCALE)/max(||f||, eps) = 1/max(||f||/C, eps/C)
            C = OUT_SCALE / KSCALE
            nc.scalar.activation(
                rfn_p[:], nf_raw[:], AF.Sqrt, bias=0.0, scale=1.0 / (C * C)
            )
            nc.vector.tensor_scalar_max(rfn_p[:], rfn_p[:], EPS / C)
            nc.vector.reciprocal(rfn_p[:], rfn_p[:])

            # ---- score GEMM (fp8 DoubleRow, frames stationary) -----------
            DR = mybir.MatmulPerfMode.DoubleRow
            NSTEP = HO // 2            # 4 contraction steps of 256
            KH = K_CHUNKS // 2         # 4 moving chunks per half
            for ft in range(F_TILES):
                fsl = slice(ft * P, (ft + 1) * P)
                for kh in range(2):
                    s_pss = [
                        psum.tile([P, NCH], F32, tag="mm", bufs=6, name="s_ps")
                        for _ in range(KH)
                    ]
                    for s in range(NSTEP):
                        lhs = f8_s[:, 2 * s:2 * s + 2, fsl]
                        for n4 in range(KH):
                            n = kh * KH + n4
                            rhs = k8_all[:, 2 * s:2 * s + 2, n * NCH:(n + 1) * NCH]
                            nc.tensor.matmul(
                                s_pss[n4][:], lhs, rhs,
                                start=(s == 0), stop=(s == NSTEP - 1),
                                perf_mode=DR,
                            )
                    for n4 in range(KH):
                        n = kh * KH + n4
                        out_t = work.tile([P, NCH], F32, tag="out_t", bufs=6,
                                          name="out_t")
                        nc.scalar.activation(
                            out_t[:], s_pss[n4][:], AF.Identity,
                            bias=bias049_s[:, 0:1], scale=rfn_p[:, ft:ft + 1],
                        )
                        nc.sync.dma_start(
                            out_d[fsl, n * NCH:(n + 1) * NCH], out_t[:]
                        )


def build():
    """Build + compile the (core-agnostic) Bass program once."""
    key = "nc_v2"
    if key in _CACHE:
        return _CACHE[key]
    nc = bacc.Bacc(
        "TRN2",
        target_bir_lowering=False,
        debug=False,
        enable_asserts=False,
        num_devices=NCORES,
    )
    featT_d = nc.dram_tensor("featT", [P, HO, FS], BF16, kind="ExternalInput").ap()
    kwT_d = nc.dram_tensor("kwT", [P, HO, KS], BF16, kind="ExternalInput").ap()
    w1t_d = nc.dram_tensor("w1t", [P, HO, H], BF16, kind="ExternalInput").ap()
    w2t_d = nc.dram_tensor("w2t", [P, HO, H], BF16, kind="ExternalInput").ap()
    wkt_d = nc.dram_tensor("wkt", [P, HO, H], BF16, kind="ExternalInput").ap()
    b1_d = nc.dram_tensor("b1t", [P, HO], F32, kind="ExternalInput").ap()
    b2_d = nc.dram_tensor("b2t", [P, HO], F32, kind="ExternalInput").ap()
    bk_d = nc.dram_tensor("bkt", [P, HO], F32, kind="ExternalInput").ap()
    out_d = nc.dram_tensor("out", [FS, K], F32, kind="ExternalOutput").ap()

    io = (featT_d, kwT_d, w1t_d, w2t_d, wkt_d, b1_d, b2_d, bk_d, out_d)
    with tile.TileContext(nc) as tc:
        _emit(tc, io)
    nc.compile()
    _CACHE[key] = nc
    return nc


def _part_tile(a):
    """[D0, rest...] with D0 = o*P + p  ->  [P, D0//P, rest...]"""
    d0 = a.shape[0]
    return np.ascontiguousarray(
        a.reshape(d0 // P, P, *a.shape[1:]).swapaxes(0, 1)
    )


def make_in_maps(feat, keyword, W1, b1, W2, b2, Wk, bk):
    bf = ml_dtypes.bfloat16
    feat = np.asarray(feat, np.float32)
    keyword = np.asarray(keyword, np.float32)
    w1t = _part_tile(np.ascontiguousarray(np.asarray(W1, np.float32).T)).astype(bf)
    w2t = _part_tile(np.ascontiguousarray(np.asarray(W2, np.float32).T)).astype(bf)
    wkt = _part_tile(np.ascontiguousarray(np.asarray(Wk, np.float32).T)).astype(bf)
    b1t = _part_tile(np.asarray(b1, np.float32))                        # [P, HO]
    b2t = _part_tile(np.asarray(b2, np.float32))
    bkt = _part_tile(np.asarray(bk, np.float32))

    in_maps = []
    for c in range(NCORES):
        featT_c = _part_tile(
            np.ascontiguousarray(feat[c * FS:(c + 1) * FS, :].T)
        ).astype(bf)                                                    # [P, HO, FS]
        kwT_c = _part_tile(
            np.ascontiguousarray(keyword[c * KS:(c + 1) * KS, :].T)
        ).astype(bf)                                                    # [P, HO, KS]
        in_maps.append({
            "featT": featT_c,
            "kwT": kwT_c,
            "w1t": w1t,
            "w2t": w2t,
            "wkt": wkt,
            "b1t": b1t,
            "b2t": b2t,
            "bkt": bkt,
        })
    return in_maps


def assemble_out(shards):
    """shards[c] is the [FS, K] transposed score tile for frames of core c."""
    return np.ascontiguousarray(
        np.concatenate([np.asarray(s).T for s in shards], axis=1)
    ).astype(np.float32)


def kernel(feat, keyword, W1, b1, W2, b2, Wk, bk, _trace=False):
    global LAST_EXEC_NS, LAST_RESULTS
    nc = build()
    in_maps = make_in_maps(feat, keyword, W1, b1, W2, b2, Wk, bk)
    res = run_bass_kernel_spmd(
        nc,
        in_maps,
        core_ids=list(range(NCORES)),
        trace=_trace,
    )
    LAST_EXEC_NS = res.exec_time_ns
    LAST_RESULTS = res
    return assemble_out([res.results[c]["out"] for c in range(NCORES)])
